# revision 1
# baseline (speedup 1.0000x reference)
"""GroupedQueryAttention TRN2 kernel: 8-way tensor-parallel over heads.

Sharding: core c gets query heads 4c..4c+3 (W_query rows 256c:256c+256),
KV head c (W_key/W_value rows 64c:64c+64), W_out columns 256c:256c+256.
x is replicated; each core computes a partial [T, C] output; host sums.

Per-core dataflow (all transposed "T-on-free" layout, f32r matmuls):
  xT (host-pretransposed) streamed in quarters -> q^T/k^T/v^T via PE
  RMSNorm via PE ones-matmul sumsq + ACT sqrt + DVE recip; RoPE via PE
  partition-swap matmul + DVE muls.  Attention per head: S^T strips
  [128k x 512q] (causal-trimmed), additive -60 mask on diag window, ACT
  exp into f32r P^T, A@V with ones-augmented V giving ctx^T and softmax
  sums in one accumulation.  ctx^T normalized via PE broadcast matmul,
  then out-proj back to natural [T, C] and DMA out.
"""

import sys

sys.path.insert(0, "/opt/trn_rl_repo")

import numpy as np

import concourse.bass as bass
import concourse.mybir as mybir
import concourse.tile as tile
from concourse import bacc
from concourse.bass_utils import run_bass_kernel_spmd

H, KV, D, EPS = 32, 8, 64, 1e-6
T = 2048
C = 2048
HPC = H // 8          # 4 query heads per core
DQ = HPC * D          # 256
W = 448               # qkv out dims per core (256 + 64 + 64dup + 64)
NW = 512              # matmul moving free dim
F32 = mybir.dt.float32
F32R = mybir.dt.float32r
AF = mybir.ActivationFunctionType

_PROG = None


def _build_program():
    nc = bacc.Bacc("TRN2", target_bir_lowering=False, debug=False)

    xt_d = nc.declare_dram_parameter("xt", [C, T], F32R, isOutput=False)
    wqkvt_d = nc.declare_dram_parameter("wqkvt", [C, W], F32R, isOutput=False)
    wot_d = nc.declare_dram_parameter("wot", [DQ, C], F32R, isOutput=False)
    cost_d = nc.declare_dram_parameter("cost", [128, T], F32, isOutput=False)
    sints_d = nc.declare_dram_parameter("sints", [128, T], F32, isOutput=False)
    cm_d = nc.declare_dram_parameter("cm", [128, 4 * NW], F32, isOutput=False)
    fa_d = nc.declare_dram_parameter("fa", [65, 128], F32R, isOutput=False)
    fb_d = nc.declare_dram_parameter("fb", [65, 128], F32R, isOutput=False)
    fk_d = nc.declare_dram_parameter("fk", [65, 128], F32R, isOutput=False)
    ea_d = nc.declare_dram_parameter("ea", [4, 128], F32R, isOutput=False)
    eb_d = nc.declare_dram_parameter("eb", [4, 128], F32R, isOutput=False)
    sqo_d = nc.declare_dram_parameter("sqo", [128, 5], F32R, isOutput=False)
    perm_d = nc.declare_dram_parameter("perm", [128, 128], F32R, isOutput=False)
    id64_d = nc.declare_dram_parameter("id64", [64, 64], F32R, isOutput=False)
    ones16_d = nc.declare_dram_parameter("ones16", [128, 16], F32R, isOutput=False)
    out_d = nc.declare_dram_parameter("out", [T, C], F32, isOutput=True)

    with tile.TileContext(nc) as tc:
        with tc.tile_pool(name="persist", bufs=1) as pp:
            qT = pp.tile([128, 2 * T], F32R, tag="qT")       # [dims(2x128), (m,t)]
            kkT = pp.tile([128, T], F32R, tag="kkT")         # k dup both halves
            vT = pp.tile([64, T], F32R, tag="vT")
            cosT = pp.tile([128, T], F32, tag="cosT")
            sinTs = pp.tile([128, T], F32, tag="sinTs")
            cm = pp.tile([128, 4 * NW], F32, tag="cm")
            fa = pp.tile([65, 128], F32R, tag="fa")
            fb = pp.tile([65, 128], F32R, tag="fb")
            fk = pp.tile([65, 128], F32R, tag="fk")
            ea = pp.tile([4, 128], F32R, tag="ea")
            eb = pp.tile([4, 128], F32R, tag="eb")
            sqo = pp.tile([128, 5], F32R, tag="sqo")
            perm = pp.tile([128, 128], F32R, tag="perm")
            id64 = pp.tile([64, 64], F32R, tag="id64")
            biasq = pp.tile([34, 1], F32, tag="biasq")
            biask = pp.tile([65, 1], F32, tag="biask")
            scalek = pp.tile([65, 1], F32, tag="scalek")
            nc.vector.memset(biasq[:], float(64 * EPS))
            nc.vector.memset(biask[:], float(EPS))
            nc.vector.memset(scalek[:], float(1.0 / 64))
            nc.sync.dma_start(cosT[:], cost_d[:])
            nc.sync.dma_start(sinTs[:], sints_d[:])
            nc.sync.dma_start(cm[:], cm_d[:])
            nc.sync.dma_start(fa[:], fa_d[:])
            nc.sync.dma_start(fb[:], fb_d[:])
            nc.sync.dma_start(fk[:], fk_d[:])
            nc.sync.dma_start(ea[:], ea_d[:])
            nc.sync.dma_start(eb[:], eb_d[:])
            nc.sync.dma_start(sqo[:], sqo_d[:])
            nc.sync.dma_start(perm[:], perm_d[:])
            nc.sync.dma_start(id64[:], id64_d[:])

            # ---------------- Phase 1: QKV projections ----------------
            with tc.tile_pool(name="p1sb", bufs=2) as p1sb, \
                 tc.tile_pool(name="p1w", bufs=1) as p1w, \
                 tc.tile_pool(name="p1ps", bufs=2, space="PSUM") as p1ps:
                wq = p1w.tile([128, 16 * W], F32R, tag="wq")
                nc.sync.dma_start(
                    wq[:].rearrange("p (c w) -> p c w", w=W),
                    wqkvt_d[:].rearrange("(c p) w -> p c w", p=128),
                )
                for qtr in range(4):
                    xq = p1sb.tile([128, 16 * NW], F32R, tag="xq")
                    nc.sync.dma_start(
                        xq[:].rearrange("p (c t) -> p c t", t=NW),
                        xt_d[:, qtr * NW:(qtr + 1) * NW].rearrange(
                            "(c p) t -> p c t", p=128),
                    )
                    pq0 = p1ps.tile([128, NW], F32, tag="pq0")
                    pq1 = p1ps.tile([128, NW], F32, tag="pq1")
                    pkk = p1ps.tile([128, NW], F32, tag="pkk")
                    pvv = p1ps.tile([64, NW], F32, tag="pvv")
                    for c in range(16):
                        st, sp = (c == 0), (c == 15)
                        nc.tensor.matmul(pq0[:], wq[:, W * c:W * c + 128],
                                         xq[:, NW * c:NW * (c + 1)],
                                         start=st, stop=sp)
                        nc.tensor.matmul(pq1[:], wq[:, W * c + 128:W * c + 256],
                                         xq[:, NW * c:NW * (c + 1)],
                                         start=st, stop=sp)
                        nc.tensor.matmul(pkk[:], wq[:, W * c + 256:W * c + 384],
                                         xq[:, NW * c:NW * (c + 1)],
                                         start=st, stop=sp)
                        nc.tensor.matmul(pvv[:], wq[:, W * c + 384:W * c + 448],
                                         xq[:, NW * c:NW * (c + 1)],
                                         start=st, stop=sp)
                    nc.vector.tensor_copy(qT[:, NW * qtr:NW * (qtr + 1)], pq0[:])
                    nc.vector.tensor_copy(qT[:, T + NW * qtr:T + NW * (qtr + 1)],
                                          pq1[:])
                    nc.vector.tensor_copy(kkT[:, NW * qtr:NW * (qtr + 1)], pkk[:])
                    nc.vector.tensor_copy(vT[:, NW * qtr:NW * (qtr + 1)], pvv[:])

            # ---------------- Phase 2: RMSNorm + RoPE ----------------
            with tc.tile_pool(name="p2sb", bufs=1) as p2sb, \
                 tc.tile_pool(name="p2tmp", bufs=2) as p2tmp, \
                 tc.tile_pool(name="p2ps", bufs=2, space="PSUM") as p2ps:
                rms = p2sb.tile([65, T], F32, tag="rms")
                rinv = p2sb.tile([65, T], F32, tag="rinv")
                rinvr = p2sb.tile([65, T], F32R, tag="rinvr")

                for m in range(2):
                    qc = qT[:, T * m:T * (m + 1)]
                    t2 = p2tmp.tile([128, T], F32R, tag="t2")
                    nc.vector.tensor_mul(t2[:], qc, qc)
                    ss = p2ps.tile([2, T], F32, tag="ps2")
                    for w in range(4):
                        nc.tensor.matmul(ss[:, NW * w:NW * (w + 1)],
                                         sqo[:, 2 * m:2 * m + 2],
                                         t2[:, NW * w:NW * (w + 1)],
                                         start=True, stop=True)
                    rr = 32 * m
                    nc.scalar.activation(rms[rr:rr + 2, :], ss[:],
                                         AF.Sqrt, bias=biasq[rr:rr + 2, :],
                                         scale=1.0)
                kc = kkT[:]
                t2k = p2tmp.tile([128, T], F32R, tag="t2")
                nc.vector.tensor_mul(t2k[0:64, :], kkT[0:64, :], kkT[0:64, :])
                ssk = p2ps.tile([1, T], F32, tag="ps2")
                for w in range(4):
                    nc.tensor.matmul(ssk[:, NW * w:NW * (w + 1)],
                                     sqo[0:64, 4:5],
                                     t2k[0:64, NW * w:NW * (w + 1)],
                                     start=True, stop=True)
                nc.scalar.activation(rms[64:65, :], ssk[:], AF.Sqrt,
                                     bias=biask[64:65, :],
                                     scale=scalek[64:65, :])
                for rr, n in ((0, 2), (32, 2), (64, 1)):
                    nc.vector.reciprocal(rinv[rr:rr + n, :], rms[rr:rr + n, :])
                    nc.vector.tensor_copy(rinvr[rr:rr + n, :], rinv[rr:rr + n, :])

                for m in range(2):
                    qc = qT[:, T * m:T * (m + 1)]
                    pb = p2ps.tile([128, T], F32, tag="ps2")
                    lhs = fa if m == 0 else fb
                    for w in range(4):
                        nc.tensor.matmul(pb[:, NW * w:NW * (w + 1)], lhs[:],
                                         rinvr[:, NW * w:NW * (w + 1)],
                                         start=True, stop=True)
                    nc.vector.tensor_mul(qc, qc, pb[:])
                    psw = p2ps.tile([128, T], F32, tag="ps2")
                    for w in range(4):
                        nc.tensor.matmul(psw[:, NW * w:NW * (w + 1)], perm[:],
                                         qc[:, NW * w:NW * (w + 1)],
                                         start=True, stop=True)
                    tm1 = p2tmp.tile([128, T], F32, tag="tm1")
                    tm2 = p2tmp.tile([128, T], F32, tag="tm2")
                    nc.vector.tensor_mul(tm1[:], qc, cosT[:])
                    nc.vector.tensor_mul(tm2[:], psw[:], sinTs[:])
                    nc.vector.tensor_add(qc, tm1[:], tm2[:])
                # k (duplicated in both partition halves of kkT)
                pbk = p2ps.tile([128, T], F32, tag="ps2")
                for w in range(4):
                    nc.tensor.matmul(pbk[:, NW * w:NW * (w + 1)], fk[:],
                                     rinvr[:, NW * w:NW * (w + 1)],
                                     start=True, stop=True)
                nc.vector.tensor_mul(kkT[:], kkT[:], pbk[:])
                pswk = p2ps.tile([128, T], F32, tag="ps2")
                for w in range(4):
                    nc.tensor.matmul(pswk[:, NW * w:NW * (w + 1)],
                                     perm[:],
                                     kkT[:, NW * w:NW * (w + 1)],
                                     start=True, stop=True)
                tm1k = p2tmp.tile([128, T], F32, tag="tm1")
                tm2k = p2tmp.tile([128, T], F32, tag="tm2")
                nc.vector.tensor_mul(tm1k[:], kkT[:], cosT[:])
                nc.vector.tensor_mul(tm2k[:], pswk[:], sinTs[:])
                nc.vector.tensor_add(kkT[:], tm1k[:], tm2k[:])

            # ---------------- Phase 3: V natural (ones-augmented) -------
            with tc.tile_pool(name="p3sb", bufs=1) as p3sb:
                vaug = p3sb.tile([128, 16 * 65], F32R, tag="vaug")
                nc.sync.dma_start(
                    vaug[:].rearrange("p (i c) -> p i c", c=65)[:, :, 64:65],
                    ones16_d[:].rearrange("p (i c) -> p i c", c=1),
                )
                with tc.tile_pool(name="p3ps", bufs=2, space="PSUM") as p3ps:
                    for i in range(16):
                        pv = p3ps.tile([128, 64], F32R, tag="pv")
                        nc.tensor.transpose(pv[:], vT[:, 128 * i:128 * (i + 1)],
                                            id64[:])
                        nc.vector.tensor_copy(vaug[:, 65 * i:65 * i + 64], pv[:])

                # ---------------- Phase 4: attention per head ----------
                recip = p3sb.tile([4, T], F32, tag="recip")
                sums4 = p3sb.tile([4, T], F32, tag="sums4")
                sums_sb = p3sb.tile([65, 4 * T], F32, tag="sums_sb")
                ctxT = p3sb.tile([128, 2 * T], F32R, tag="ctxT")
                tmpc = p3sb.tile([64, T], F32R, tag="tmpc")
                with tc.tile_pool(name="p4pt", bufs=2) as p4pt, \
                     tc.tile_pool(name="p4s", bufs=1, space="PSUM") as p4s, \
                     tc.tile_pool(name="p4c", bufs=1, space="PSUM") as p4c:
                    for h in range(HPC):
                        qh = qT[64 * (h % 2):64 * (h % 2) + 64,
                                T * (h // 2):T * (h // 2 + 1)]
                        ctx = p4c.tile([65, T], F32, tag="ctx")
                        for i in range(16):
                            j0 = i // 4
                            r = i % 4
                            wdt = (4 - j0) * NW
                            s_ps = p4s.tile([128, T], F32, tag="s")
                            for j in range(j0, 4):
                                nc.tensor.matmul(
                                    s_ps[:, NW * (j - j0):NW * (j - j0 + 1)],
                                    kkT[64 * (h % 2):64 * (h % 2) + 64,
                                        128 * i:128 * (i + 1)],
                                    qh[:, NW * j:NW * (j + 1)],
                                    start=True, stop=True)
                            nc.vector.tensor_add(s_ps[:, 0:NW], s_ps[:, 0:NW],
                                                 cm[:, NW * r:NW * (r + 1)])
                            pt = p4pt.tile([128, T], F32R, tag="pt")
                            nc.scalar.activation(pt[:, 0:wdt], s_ps[:, 0:wdt],
                                                 AF.Exp)
                            for j in range(j0, 4):
                                nc.tensor.matmul(
                                    ctx[:, NW * j:NW * (j + 1)],
                                    vaug[:, 65 * i:65 * (i + 1)],
                                    pt[:, NW * (j - j0):NW * (j - j0 + 1)],
                                    start=(i == 0), stop=(i == 4 * j + 3))
                        nc.vector.tensor_copy(sums_sb[64:65, T * h:T * (h + 1)],
                                               ctx[64:65, :])
                        if h % 2 == 0:
                            nc.vector.tensor_copy(
                                ctxT[0:64, T * (h // 2):T * (h // 2 + 1)],
                                ctx[0:64, :])
                        else:
                            nc.vector.tensor_copy(tmpc[:], ctx[0:64, :])
                            nc.sync.dma_start(
                                ctxT[64:128, T * (h // 2):T * (h // 2 + 1)],
                                tmpc[:])
                    for h in range(HPC):
                        nc.sync.dma_start(sums4[h:h + 1, :],
                                          sums_sb[64:65, T * h:T * (h + 1)])
                    nc.vector.reciprocal(recip[:], sums4[:])

                # ------------- Phase 5: normalize + out-proj -----------
                rinvc = p3sb.tile([4, T], F32R, tag="rinvc")
                nc.vector.tensor_copy(rinvc[:], recip[:])
                with tc.tile_pool(name="p5w", bufs=1) as p5w, \
                     tc.tile_pool(name="p5o", bufs=3) as p5o, \
                     tc.tile_pool(name="p5ps", bufs=2, space="PSUM") as p5ps:
                    wo = p5w.tile([128, 2 * T], F32R, tag="wo")
                    nc.sync.dma_start(
                        wo[:].rearrange("p (m t) -> p m t", t=T),
                        wot_d[:].rearrange("(m p) t -> p m t", p=128),
                    )
                    for m in range(2):
                        cc = ctxT[:, T * m:T * (m + 1)]
                        pb2 = p5ps.tile([128, T], F32, tag="po")
                        lhs = ea if m == 0 else eb
                        for w in range(4):
                            nc.tensor.matmul(pb2[:, NW * w:NW * (w + 1)],
                                             lhs[:], rinvc[:, NW * w:NW * (w + 1)],
                                             start=True, stop=True)
                        nc.vector.tensor_mul(cc, cc, pb2[:])
                    for t in range(16):
                        po = p5ps.tile([128, T], F32, tag="po")
                        for m2 in range(2):
                            for w in range(4):
                                nc.tensor.matmul(
                                    po[:, NW * w:NW * (w + 1)],
                                    ctxT[:, T * m2 + 128 * t:T * m2 + 128 * (t + 1)],
                                    wo[:, T * m2 + NW * w:T * m2 + NW * (w + 1)],
                                    start=(m2 == 0), stop=(m2 == 1))
                        ot = p5o.tile([128, T], F32, tag="ot")
                        nc.vector.tensor_copy(ot[:], po[:])
                        nc.sync.dma_start(out_d[128 * t:128 * (t + 1), :], ot[:])

    nc.compile()
    return nc


def _host_constants():
    iv = 1.0 / (10000.0 ** (np.arange(0, D, 2, dtype=np.float32) / D))
    ang = np.arange(T, dtype=np.float32)[:, None] * iv[None, :]
    ang = np.concatenate([ang, ang], axis=-1)          # [T, 64]
    return np.cos(ang), np.sin(ang)


def kernel(x, mask, cos, sin, W_query, W_key, W_value, W_out,
           q_norm_w, k_norm_w):
    global _PROG
    if _PROG is None:
        _PROG = _build_program()
    nc = _PROG

    x = np.asarray(x, np.float32)
    cos = np.asarray(cos, np.float32)
    sin = np.asarray(sin, np.float32)
    W_query = np.asarray(W_query, np.float32)
    W_key = np.asarray(W_key, np.float32)
    W_value = np.asarray(W_value, np.float32)
    W_out = np.asarray(W_out, np.float32)
    q_norm_w = np.asarray(q_norm_w, np.float32)
    k_norm_w = np.asarray(k_norm_w, np.float32)

    xt = np.ascontiguousarray(x[0].T)                  # [C, T]
    cosT1 = np.ascontiguousarray(cos[:T].T)            # [64, T]
    sinT1 = np.ascontiguousarray(sin[:T].T).copy()
    sinT1[0:32, :] *= -1.0                             # signed for rotate-half
    cosT = np.concatenate([cosT1, cosT1], axis=0)      # [128, T]
    sinT = np.concatenate([sinT1, sinT1], axis=0)

    # causal mask tiles for the diagonal 512-window of each k-strip
    p = np.arange(128)[:, None]
    col = np.arange(4 * NW)[None, :]
    cmf = np.zeros((128, 4 * NW), np.float32)
    for r in range(4):
        cw = col[:, NW * r:NW * (r + 1)] - NW * r
        cmf[:, NW * r:NW * (r + 1)] = np.where(cw < 128 * r + p, -60.0, 0.0)

    fa = np.zeros((65, 128), np.float32)
    fb = np.zeros((65, 128), np.float32)
    fk = np.zeros((65, 128), np.float32)
    fa[0, 0:64] = q_norm_w
    fa[1, 64:128] = q_norm_w
    fb[32, 0:64] = q_norm_w
    fb[33, 64:128] = q_norm_w
    fk[64, 0:64] = k_norm_w
    fk[64, 64:128] = k_norm_w
    ea = np.zeros((4, 128), np.float32)
    eb = np.zeros((4, 128), np.float32)
    ea[0, 0:64] = 1.0
    ea[1, 64:128] = 1.0
    eb[2, 0:64] = 1.0
    eb[3, 64:128] = 1.0
    sqo = np.zeros((128, 5), np.float32)
    sqo[0:64, 0] = 1.0
    sqo[64:128, 1] = 1.0
    sqo[0:64, 2] = 1.0
    sqo[64:128, 3] = 1.0
    sqo[0:64, 4] = 1.0
    perm = np.zeros((128, 128), np.float32)
    for b in range(2):
        for d in range(64):
            perm[64 * b + (d ^ 32), 64 * b + d] = 1.0
    id64 = np.eye(64, dtype=np.float32)

    shared = {
        "xt": xt, "cost": cosT, "sints": sinT, "cm": cmf,
        "fa": fa, "fb": fb, "fk": fk, "ea": ea, "eb": eb,
        "sqo": sqo, "perm": perm, "id64": id64,
        "ones16": np.ones((128, 16), np.float32),
    }
    in_maps = []
    for c in range(8):
        wqkvt = np.ascontiguousarray(np.concatenate(
            [W_query[DQ * c:DQ * (c + 1)],
             W_key[64 * c:64 * (c + 1)],
             W_key[64 * c:64 * (c + 1)],
             W_value[64 * c:64 * (c + 1)]], axis=0).T)   # [C, 448]
        wot = np.ascontiguousarray(W_out[:, DQ * c:DQ * (c + 1)].T)  # [256, C]
        in_maps.append(dict(shared, wqkvt=wqkvt, wot=wot))

    res = run_bass_kernel_spmd(nc, in_maps, list(range(8)))
    out = res.results[0]["out"].astype(np.float64)
    for c in range(1, 8):
        out += res.results[c]["out"]
    return out.astype(np.float32)[None]

